# revision 26
# baseline (speedup 1.0000x reference)
"""Trainium2 Bass kernel for CrossAttention (B=2, C=128, H=W=64, heads=4, d=64).

Sharding: one (batch, head) pair per NeuronCore (2*4 = 8 cores).

Per-core computation (all on device):
  q = Wq_h @ x_b            [64, 4096]
  k = Wk_h @ y_b            [64, 4096]
  vT = (y_b.T @ WvT_h)      [4096, 64]   (produced transposed, j on partitions)
  row-l2-normalize q, k along the 4096 axis (both norms folded into k as
        g[d] = 1/(||q_d|| * ||k_d||))
  sim_T[j, i] = sum_d k[d, j] q[d, i]    (transposed attention logits)
  e_T = exp(SCALE * sim_T)
  outT_aug[m, i] = sum_j vT_aug[j, m] * e_T[j, i]  where vT_aug has a ones
        column at m=64, so row 64 accumulates the softmax denominator l[i].
  outT = outT_aug[0:64] / l   (partition-broadcast of 1/l)
  partial[o, i] = WoT_h.T @ outT          [128, 4096]
Host: out[b] = sum_h partial[b, h] + bias; reshape to [2, 128, 64, 64].

Logits are bounded (|sim| <= 10 by Cauchy-Schwarz, ~0.4 in practice), so the
softmax max-subtraction is skipped: exp() cannot overflow.

The attention matmuls run in float32r (full PE rate; ~11-bit mantissa).
Overall rel err vs the fp32 reference is ~1.5e-4.
"""

import numpy as np

import concourse.bacc as bacc
import concourse.mybir as mybir
from concourse.bass import ts, ds
from concourse.tile import TileContext
from concourse.bass_utils import run_bass_kernel_spmd

F32 = mybir.dt.float32
F32R = mybir.dt.float32r

B, C, HW = 2, 128, 4096
HEADS, D = 4, 64
HIDDEN = HEADS * D
SCALE = 10.0
N_CORES = 8

IC = 1024          # i-axis chunk per outer iteration (PSUM-bank limited)
N_IC = HW // IC    # 4
N_J = HW // 128    # 32 j-tiles of 128
NCH = HW // 512    # 8 projection chunks


def _emit_one(nc, tc, io, mm_fast, rep):
    """Emit one full forward pass. rep distinguishes pool names across repeats."""
    xb, yb, wqT, wkT, wvT, woT, outp = io
    MMDT = F32R if mm_fast else F32
    Exp = mybir.ActivationFunctionType.Exp
    Square = mybir.ActivationFunctionType.Square
    mult = mybir.AluOpType.mult

    with tc.tile_pool(name=f"big{rep}", bufs=1) as big, \
         tc.tile_pool(name=f"const{rep}", bufs=1) as const:
        wq_sb = const.tile([C, D], F32)
        wk_sb = const.tile([C, D], F32)
        wv_sb = const.tile([C, D], F32)
        wo_sb = const.tile([D, C], F32)
        # weights first: tiny DMAs that gate the first projection matmuls
        nc.sync.dma_start(wq_sb, wqT[:])
        nc.sync.dma_start(wk_sb, wkT[:])
        nc.sync.dma_start(wv_sb, wvT[:])
        nc.sync.dma_start(wo_sb, woT[:])

        # Inputs are DMA'd straight into float32r-typed tiles (byte-identical
        # bitcast view): the PE's fp32r operand path rounds on read, so no
        # explicit cast pass is needed.
        x_r = big.tile([C, HW], MMDT)
        y_r = big.tile([C, HW], MMDT)
        # chunked input DMA so projections start before the full load lands;
        # xb on the SP HWDGE queue, yb on the ACT HWDGE queue (parallel)
        for t in range(NCH):
            nc.sync.dma_start(x_r[:, ts(t, 512)], xb[:, ts(t, 512)].bitcast(MMDT))
            nc.scalar.dma_start(y_r[:, ts(t, 512)], yb[:, ts(t, 512)].bitcast(MMDT))

        wq_r = const.tile([C, D], MMDT)
        wk_r = const.tile([C, D], MMDT)
        wv_r = const.tile([C, D], MMDT)
        q_r = big.tile([D, HW], MMDT)
        k_sb = big.tile([D, HW], F32)
        k_r = big.tile([D, HW], MMDT)
        vT_sb = big.tile([128, N_J, D + 1], MMDT)
        outT_sb = big.tile([D + 1, HW], F32)
        out_sb = big.tile([C, HW], F32)
        scr = big.tile([D, 512], F32)
        qparts = const.tile([D, NCH], F32)
        kparts = const.tile([D, NCH], F32)
        ones_sb = const.tile([128, 1], F32)
        nc.vector.memset(ones_sb, 1.0)
        ones64 = const.tile([1, D], F32)
        nc.vector.memset(ones64, 1.0)

        # ---------------- Stage A: projections + normalization ----------
        with tc.tile_pool(name=f"psA{rep}", bufs=2, space="PSUM") as psA:
            # PE warm-up: dense dummy matmuls during the DMA window nudge the
            # HAM clock gate toward 2.4 GHz before the real work lands.
            warm = const.tile([128, 512], F32)
            nc.vector.memset(warm, 0.0)
            for w in range(2):
                pw = psA.tile([128, 512], F32, tag="pw")
                nc.tensor.matmul(pw, lhsT=warm[:, 0:128], rhs=warm[:],
                                 start=True, stop=True)
            nc.vector.tensor_copy(wq_r[:], wq_sb[:])
            nc.vector.tensor_copy(wk_r[:], wk_sb[:])
            nc.vector.tensor_copy(wv_r[:], wv_sb[:])
            for t in range(NCH):
                pq = psA.tile([D, 512], F32, tag="pq")
                nc.tensor.matmul(pq, lhsT=wq_r[:], rhs=x_r[:, ts(t, 512)],
                                 start=True, stop=True)
                nc.vector.tensor_copy(q_r[:, ts(t, 512)], pq)
                # sum of squares of this chunk via ACT (idle in stage A):
                # scr = pq^2, accum_out = row-sum(scr)
                nc.scalar.activation(scr, pq, Square,
                                     accum_out=qparts[:, t:t + 1])
                pk = psA.tile([D, 512], F32, tag="pk")
                nc.tensor.matmul(pk, lhsT=wk_r[:], rhs=y_r[:, ts(t, 512)],
                                 start=True, stop=True)
                nc.vector.tensor_copy(k_sb[:, ts(t, 512)], pk)
                nc.scalar.activation(scr, pk, Square,
                                     accum_out=kparts[:, t:t + 1])
                # vT blocks for this yb chunk: (yb 128-col).T @ wv -> [128, 64]
                for j in range(4 * t, 4 * t + 4):
                    pv = psA.tile([128, D], F32, tag="pv")
                    nc.tensor.matmul(pv, lhsT=y_r[:, ts(j, 128)],
                                     rhs=wv_r[:], start=True, stop=True)
                    nc.vector.tensor_copy(vT_sb[:, j, 0:D], pv)
            nc.vector.tensor_copy(vT_sb[:, :, D],
                                  ones_sb.to_broadcast((128, N_J)))

        # g[d] = 1/sqrt(ssq_q[d] * ssq_k[d]), computed entirely on DVE so the
        # ACT table set (exp_and_others, loaded once for the Squares) never
        # switches mid-kernel: bit-trick rsqrt seed + 3 Newton iterations.
        ssq_q = const.tile([D, 1], F32)
        ssq_k = const.tile([D, 1], F32)
        nc.vector.reduce_sum(ssq_q, qparts[:], axis=mybir.AxisListType.X)
        nc.vector.reduce_sum(ssq_k, kparts[:], axis=mybir.AxisListType.X)
        P = const.tile([D, 1], F32)
        nc.vector.tensor_mul(P, ssq_q, ssq_k)
        nc.vector.tensor_scalar_max(P, P, 1e-24)
        hi = const.tile([D, 1], mybir.dt.int32)
        nc.vector.tensor_scalar(hi, P.bitcast(mybir.dt.int32), 1, None,
                                op0=mybir.AluOpType.arith_shift_right)
        # 0x5f3759df - h == (h ^ 0xffffffff) + 0x5f3759e0
        nc.vector.tensor_scalar(hi, hi, -1, None,
                                op0=mybir.AluOpType.bitwise_xor)
        nc.vector.tensor_scalar(hi, hi, 0x5F3759E0, None,
                                op0=mybir.AluOpType.add)
        g = const.tile([D, 1], F32)
        gt = const.tile([D, 1], F32)
        y = hi.bitcast(F32)
        for it in range(3):
            src = y if it == 0 else g
            nc.vector.tensor_mul(gt, src, src)                       # y^2
            nc.vector.scalar_tensor_tensor(gt, gt, -0.5, P,
                                           op0=mult, op1=mult)       # -.5Py^2
            nc.vector.scalar_tensor_tensor(g, gt, 1.5, src,
                                           op0=mybir.AluOpType.add,
                                           op1=mult)                 # refined
        # fold both norms into k (chunked so the first QK tiles start early)
        for c4 in range(N_IC):
            nc.vector.tensor_scalar(k_r[:, ts(c4, IC)], k_sb[:, ts(c4, IC)],
                                    g, None, op0=mult)

        # ------- Main loop: flash attention (transposed) + fused epilogue ---
        with tc.tile_pool(name=f"qkps{rep}", bufs=2, space="PSUM") as qkps_pool, \
             tc.tile_pool(name=f"pvps{rep}", bufs=1, space="PSUM") as pvps_pool, \
             tc.tile_pool(name=f"epps{rep}", bufs=2, space="PSUM") as epps_pool, \
             tc.tile_pool(name=f"et{rep}", bufs=4) as e_pool, \
             tc.tile_pool(name=f"ep{rep}", bufs=2) as ep_pool:
            LOOKAHEAD = 2
            TOT = N_IC * N_J
            qk_tiles = {}
            pv_tiles = {}

            def emit_qk(jj):
                ic, j = divmod(jj, N_J)
                qk_ps = qkps_pool.tile([128, IC], F32, tag="qk_ps")
                for h2 in range(IC // 512):
                    nc.tensor.matmul(
                        qk_ps[:, ts(h2, 512)],
                        lhsT=k_r[:, ts(j, 128)],
                        rhs=q_r[:, ds(ic * IC + h2 * 512, 512)],
                        start=True, stop=True)
                qk_tiles[jj] = qk_ps

            def epilogue(ic):
                # normalize + project this i-chunk (overlaps next chunk's work)
                pv_ps = pv_tiles.pop(ic)
                oc = ds(ic * IC, IC)
                nc.vector.tensor_copy(outT_sb[:, oc], pv_ps)
                rl = ep_pool.tile([1, IC], F32, tag="rl")
                nc.vector.reciprocal(rl, outT_sb[D:D + 1, oc])
                for h2 in range(IC // 512):
                    t = ic * (IC // 512) + h2
                    # broadcast 1/l across partitions via a rank-1 matmul
                    bc = epps_pool.tile([D, 512], F32, tag="po")
                    nc.tensor.matmul(bc, lhsT=ones64[:],
                                     rhs=rl[:, ts(h2, 512)],
                                     start=True, stop=True)
                    nc.vector.tensor_mul(outT_sb[0:D, ts(t, 512)],
                                         outT_sb[0:D, ts(t, 512)], bc)
                    po = epps_pool.tile([C, 512], F32, tag="po")
                    nc.tensor.matmul(po, lhsT=wo_sb[:],
                                     rhs=outT_sb[0:D, ts(t, 512)],
                                     start=True, stop=True)
                    nc.vector.tensor_copy(out_sb[:, ts(t, 512)], po)
                    nc.sync.dma_start(outp[:, ts(t, 512)], out_sb[:, ts(t, 512)])

            for jj in range(LOOKAHEAD):
                emit_qk(jj)
            for jj in range(TOT):
                ic, j = divmod(jj, N_J)
                if jj + LOOKAHEAD < TOT:
                    emit_qk(jj + LOOKAHEAD)  # keep PE ahead of ACT
                if j == 0:
                    pv_ps = pvps_pool.tile([D + 1, IC], F32, tag="pv_ps")
                    pv_tiles[ic] = pv_ps
                qk_ps = qk_tiles.pop(jj)
                eT = e_pool.tile([128, IC], MMDT, tag="eT")
                nc.scalar.activation(eT, qk_ps, Exp, scale=SCALE)
                for h2 in range(IC // 512):
                    nc.tensor.matmul(
                        pv_tiles[ic][:, ts(h2, 512)],
                        lhsT=vT_sb[:, j, :],
                        rhs=eT[:, ts(h2, 512)],
                        start=(j == 0), stop=(j == N_J - 1))
                if j == N_J - 1:
                    epilogue(ic)


def build_nc(mm_fast=True, repeat=1):
    nc = bacc.Bacc(None, target_bir_lowering=False)
    xb = nc.dram_tensor("xb", [C, HW], F32, kind="ExternalInput")
    yb = nc.dram_tensor("yb", [C, HW], F32, kind="ExternalInput")
    wqT = nc.dram_tensor("wqT", [C, D], F32, kind="ExternalInput")
    wkT = nc.dram_tensor("wkT", [C, D], F32, kind="ExternalInput")
    wvT = nc.dram_tensor("wvT", [C, D], F32, kind="ExternalInput")
    woT = nc.dram_tensor("woT", [D, C], F32, kind="ExternalInput")
    outp = nc.dram_tensor("outp", [C, HW], F32, kind="ExternalOutput")
    io = (xb, yb, wqT, wkT, wvT, woT, outp)
    with TileContext(nc) as tc:
        for rep in range(repeat):
            _emit_one(nc, tc, io, mm_fast, rep)
    nc.finalize()
    return nc


_NC_CACHE = {}


def _get_nc(mm_fast=True, repeat=1):
    key = (mm_fast, repeat)
    if key not in _NC_CACHE:
        _NC_CACHE[key] = build_nc(mm_fast, repeat)
    return _NC_CACHE[key]


def make_in_maps(x, y, W_qkv, W_out):
    x = np.asarray(x, np.float32).reshape(B, C, HW)
    y = np.asarray(y, np.float32).reshape(B, C, HW)
    W_qkv = np.asarray(W_qkv, np.float32)
    W_out = np.asarray(W_out, np.float32)
    in_maps = []
    for core in range(N_CORES):
        b, h = core // HEADS, core % HEADS
        sl = slice(h * D, (h + 1) * D)
        in_maps.append({
            "xb": np.ascontiguousarray(x[b]),
            "yb": np.ascontiguousarray(y[b]),
            "wqT": np.ascontiguousarray(W_qkv[sl, :].T),
            "wkT": np.ascontiguousarray(W_qkv[HIDDEN + h * D:HIDDEN + (h + 1) * D, :].T),
            "wvT": np.ascontiguousarray(W_qkv[2 * HIDDEN + h * D:2 * HIDDEN + (h + 1) * D, :].T),
            "woT": np.ascontiguousarray(W_out[:, sl].T),
        })
    return in_maps


def gather(results, b_out):
    b_out = np.asarray(b_out, np.float32)
    out = np.zeros((B, C, HW), np.float32)
    for core in range(N_CORES):
        out[core // HEADS] += results[core]["outp"]
    out += b_out[None, :, None]
    return out.reshape(B, C, 64, 64)


def kernel(x, y, W_qkv, W_out, b_out):
    nc = _get_nc(mm_fast=True)
    in_maps = make_in_maps(x, y, W_qkv, W_out)
    res = run_bass_kernel_spmd(nc, in_maps, core_ids=list(range(N_CORES)))
    return gather(res.results, b_out)


# revision 28
# speedup vs baseline: 1.2681x; 1.2681x over previous
"""Trainium2 Bass kernel for CrossAttention (B=2, C=128, H=W=64, heads=4, d=64).

Sharding: one (batch, head) pair per NeuronCore (2*4 = 8 cores).

Per-core computation (all on device):
  q = Wq_h @ x_b            [64, 4096]
  k = Wk_h @ y_b            [64, 4096]
  vT = (y_b.T @ WvT_h)      [4096, 64]   (produced transposed, j on partitions)
  row-l2-normalize q, k along the 4096 axis (both norms folded into k as
        g[d] = 1/(||q_d|| * ||k_d||))
  sim_T[j, i] = sum_d k[d, j] q[d, i]    (transposed attention logits)
  e_T = exp(SCALE * sim_T)
  outT_aug[m, i] = sum_j vT_aug[j, m] * e_T[j, i]  where vT_aug has a ones
        column at m=64, so row 64 accumulates the softmax denominator l[i].
  outT = outT_aug[0:64] / l   (partition-broadcast of 1/l)
  partial[o, i] = WoT_h.T @ outT          [128, 4096]
Host: out[b] = sum_h partial[b, h] + bias; reshape to [2, 128, 64, 64].

Logits are bounded (|sim| <= 10 by Cauchy-Schwarz, ~0.4 in practice), so the
softmax max-subtraction is skipped: exp() cannot overflow.

The attention matmuls run in float32r (full PE rate; ~11-bit mantissa).
Overall rel err vs the fp32 reference is ~1.5e-4.
"""

import numpy as np

import concourse.bacc as bacc
import concourse.mybir as mybir
from concourse.bass import ts, ds
from concourse.tile import TileContext
from concourse.bass_utils import run_bass_kernel_spmd

F32 = mybir.dt.float32
F32R = mybir.dt.float32r

B, C, HW = 2, 128, 4096
HEADS, D = 4, 64
HIDDEN = HEADS * D
SCALE = 10.0
N_CORES = 8

IC = 1024          # i-axis chunk per outer iteration (PSUM-bank limited)
N_IC = HW // IC    # 4
N_J = HW // 128    # 32 j-tiles of 128
NCH = HW // 512    # 8 projection chunks


def _emit_one(nc, tc, io, mm_fast, rep):
    """Emit one full forward pass. rep distinguishes pool names across repeats."""
    xb, yb, wqT, wkT, wvT, woT, outp = io
    MMDT = F32R if mm_fast else F32
    Exp = mybir.ActivationFunctionType.Exp
    Square = mybir.ActivationFunctionType.Square
    mult = mybir.AluOpType.mult

    with tc.tile_pool(name=f"big{rep}", bufs=1) as big, \
         tc.tile_pool(name=f"const{rep}", bufs=1) as const:
        wq_sb = const.tile([C, D], F32)
        wk_sb = const.tile([C, D], F32)
        wv_sb = const.tile([C, D], F32)
        wo_sb = const.tile([D, C], F32)
        # weights first: tiny DMAs that gate the first projection matmuls
        nc.sync.dma_start(wq_sb, wqT[:])
        nc.sync.dma_start(wk_sb, wkT[:])
        nc.sync.dma_start(wv_sb, wvT[:])
        nc.sync.dma_start(wo_sb, woT[:])

        # Inputs are DMA'd straight into float32r-typed tiles (byte-identical
        # bitcast view): the PE's fp32r operand path rounds on read, so no
        # explicit cast pass is needed.
        x_r = big.tile([C, HW], MMDT)
        y_r = big.tile([C, HW], MMDT)
        # chunked input DMA so projections start before the full load lands;
        # xb on the SP HWDGE queue, yb on the ACT HWDGE queue (parallel)
        for t in range(NCH):
            nc.sync.dma_start(x_r[:, ts(t, 512)], xb[:, ts(t, 512)].bitcast(MMDT))
            nc.scalar.dma_start(y_r[:, ts(t, 512)], yb[:, ts(t, 512)].bitcast(MMDT))

        wq_r = const.tile([C, D], MMDT)
        wk_r = const.tile([C, D], MMDT)
        wv_r = const.tile([C, D], MMDT)
        q_r = big.tile([D, HW], MMDT)
        k_sb = big.tile([D, HW], F32)
        k_r = big.tile([D, HW], MMDT)
        vT_sb = big.tile([128, N_J, D + 1], MMDT)
        outT_sb = big.tile([D + 1, HW], F32)
        out_sb = big.tile([C, HW], F32)
        scr = big.tile([D, 512], F32)
        qparts = const.tile([D, NCH], F32)
        kparts = const.tile([D, NCH], F32)
        ones_sb = const.tile([128, 1], F32)
        nc.vector.memset(ones_sb, 1.0)
        ones64 = const.tile([1, D], F32)
        nc.vector.memset(ones64, 1.0)

        # ---------------- Stage A: projections + normalization ----------
        with tc.tile_pool(name=f"psA{rep}", bufs=2, space="PSUM") as psA:
            # PE warm-up: dense dummy matmuls during the DMA window nudge the
            # HAM clock gate toward 2.4 GHz before the real work lands.
            warm = const.tile([128, 512], F32)
            nc.vector.memset(warm, 0.0)
            for w in range(2):
                pw = psA.tile([128, 512], F32, tag="pw")
                nc.tensor.matmul(pw, lhsT=warm[:, 0:128], rhs=warm[:],
                                 start=True, stop=True)
            nc.vector.tensor_copy(wq_r[:], wq_sb[:])
            nc.vector.tensor_copy(wk_r[:], wk_sb[:])
            nc.vector.tensor_copy(wv_r[:], wv_sb[:])
            for t in range(NCH):
                pq = psA.tile([D, 512], F32, tag="pq")
                nc.tensor.matmul(pq, lhsT=wq_r[:], rhs=x_r[:, ts(t, 512)],
                                 start=True, stop=True)
                nc.vector.tensor_copy(q_r[:, ts(t, 512)], pq)
                # sum of squares of this chunk via ACT (idle in stage A):
                # scr = pq^2, accum_out = row-sum(scr)
                nc.scalar.activation(scr, pq, Square,
                                     accum_out=qparts[:, t:t + 1])
                pk = psA.tile([D, 512], F32, tag="pk")
                nc.tensor.matmul(pk, lhsT=wk_r[:], rhs=y_r[:, ts(t, 512)],
                                 start=True, stop=True)
                nc.vector.tensor_copy(k_sb[:, ts(t, 512)], pk)
                nc.scalar.activation(scr, pk, Square,
                                     accum_out=kparts[:, t:t + 1])
                # vT blocks for this yb chunk: (yb 128-col).T @ wv -> [128, 64]
                for j in range(4 * t, 4 * t + 4):
                    pv = psA.tile([128, D], F32, tag="pv")
                    nc.tensor.matmul(pv, lhsT=y_r[:, ts(j, 128)],
                                     rhs=wv_r[:], start=True, stop=True)
                    nc.vector.tensor_copy(vT_sb[:, j, 0:D], pv)
            nc.vector.tensor_copy(vT_sb[:, :, D],
                                  ones_sb.to_broadcast((128, N_J)))

        # g[d] = 1/sqrt(ssq_q[d] * ssq_k[d]), computed entirely on DVE so the
        # ACT table set (exp_and_others, loaded once for the Squares) never
        # switches mid-kernel: bit-trick rsqrt seed + 3 Newton iterations.
        ssq_q = const.tile([D, 1], F32)
        ssq_k = const.tile([D, 1], F32)
        nc.vector.reduce_sum(ssq_q, qparts[:], axis=mybir.AxisListType.X)
        nc.vector.reduce_sum(ssq_k, kparts[:], axis=mybir.AxisListType.X)
        P = const.tile([D, 1], F32)
        nc.vector.tensor_mul(P, ssq_q, ssq_k)
        nc.vector.tensor_scalar_max(P, P, 1e-24)
        hi = const.tile([D, 1], mybir.dt.int32)
        nc.vector.tensor_scalar(hi, P.bitcast(mybir.dt.int32), 1, None,
                                op0=mybir.AluOpType.arith_shift_right)
        # 0x5f3759df - h == (h ^ 0xffffffff) + 0x5f3759e0
        nc.vector.tensor_scalar(hi, hi, -1, None,
                                op0=mybir.AluOpType.bitwise_xor)
        nc.vector.tensor_scalar(hi, hi, 0x5F3759E0, None,
                                op0=mybir.AluOpType.add)
        g = const.tile([D, 1], F32)
        gt = const.tile([D, 1], F32)
        y = hi.bitcast(F32)
        for it in range(3):
            src = y if it == 0 else g
            nc.vector.tensor_mul(gt, src, src)                       # y^2
            nc.vector.scalar_tensor_tensor(gt, gt, -0.5, P,
                                           op0=mult, op1=mult)       # -.5Py^2
            nc.vector.scalar_tensor_tensor(g, gt, 1.5, src,
                                           op0=mybir.AluOpType.add,
                                           op1=mult)                 # refined
        # fold both norms into k (chunked so the first QK tiles start early)
        for c4 in range(N_IC):
            nc.vector.tensor_scalar(k_r[:, ts(c4, IC)], k_sb[:, ts(c4, IC)],
                                    g, None, op0=mult)

        # ------- Main loop: flash attention (transposed) + fused epilogue ---
        with tc.tile_pool(name=f"qkps{rep}", bufs=3, space="PSUM") as qkps_pool, \
             tc.tile_pool(name=f"pvps{rep}", bufs=1, space="PSUM") as pvps_pool, \
             tc.tile_pool(name=f"et{rep}", bufs=6) as e_pool, \
             tc.tile_pool(name=f"ep{rep}", bufs=2) as ep_pool:
            LOOKAHEAD = 3
            TOT = N_IC * N_J
            qk_tiles = {}
            pv_tiles = {}

            def emit_qk(jj):
                ic, j = divmod(jj, N_J)
                qk_ps = qkps_pool.tile([128, IC], F32, tag="qk_ps")
                for h2 in range(IC // 512):
                    nc.tensor.matmul(
                        qk_ps[:, ts(h2, 512)],
                        lhsT=k_r[:, ts(j, 128)],
                        rhs=q_r[:, ds(ic * IC + h2 * 512, 512)],
                        start=True, stop=True)
                qk_tiles[jj] = qk_ps

            def epilogue(ic):
                # normalize + project this i-chunk (overlaps next chunk's work)
                pv_ps = pv_tiles.pop(ic)
                oc = ds(ic * IC, IC)
                nc.vector.tensor_copy(outT_sb[:, oc], pv_ps)
                rl = ep_pool.tile([1, IC], F32, tag="rl")
                nc.vector.reciprocal(rl, outT_sb[D:D + 1, oc])
                for h2 in range(IC // 512):
                    t = ic * (IC // 512) + h2
                    # epilogue PSUM borrows a QK-pool slot: 1/l broadcast
                    # (rank-1 matmul) in the first bank, projection in the
                    # second.
                    ep_ps = qkps_pool.tile([128, IC], F32, tag="qk_ps")
                    bc = ep_ps[0:D, 0:512]
                    nc.tensor.matmul(bc, lhsT=ones64[:],
                                     rhs=rl[:, ts(h2, 512)],
                                     start=True, stop=True)
                    nc.vector.tensor_mul(outT_sb[0:D, ts(t, 512)],
                                         outT_sb[0:D, ts(t, 512)], bc)
                    po = ep_ps[:, 512:1024]
                    nc.tensor.matmul(po, lhsT=wo_sb[:],
                                     rhs=outT_sb[0:D, ts(t, 512)],
                                     start=True, stop=True)
                    nc.vector.tensor_copy(out_sb[:, ts(t, 512)], po)
                    nc.sync.dma_start(outp[:, ts(t, 512)], out_sb[:, ts(t, 512)])

            for jj in range(LOOKAHEAD):
                emit_qk(jj)
            for jj in range(TOT):
                ic, j = divmod(jj, N_J)
                if jj + LOOKAHEAD < TOT:
                    emit_qk(jj + LOOKAHEAD)  # keep PE ahead of ACT
                if j == 0:
                    pv_ps = pvps_pool.tile([D + 1, IC], F32, tag="pv_ps")
                    pv_tiles[ic] = pv_ps
                qk_ps = qk_tiles.pop(jj)
                eT = e_pool.tile([128, IC], MMDT, tag="eT")
                nc.scalar.activation(eT, qk_ps, Exp, scale=SCALE)
                for h2 in range(IC // 512):
                    nc.tensor.matmul(
                        pv_tiles[ic][:, ts(h2, 512)],
                        lhsT=vT_sb[:, j, :],
                        rhs=eT[:, ts(h2, 512)],
                        start=(j == 0), stop=(j == N_J - 1))
                if j == N_J - 1:
                    epilogue(ic)


def build_nc(mm_fast=True, repeat=1):
    nc = bacc.Bacc(None, target_bir_lowering=False)
    xb = nc.dram_tensor("xb", [C, HW], F32, kind="ExternalInput")
    yb = nc.dram_tensor("yb", [C, HW], F32, kind="ExternalInput")
    wqT = nc.dram_tensor("wqT", [C, D], F32, kind="ExternalInput")
    wkT = nc.dram_tensor("wkT", [C, D], F32, kind="ExternalInput")
    wvT = nc.dram_tensor("wvT", [C, D], F32, kind="ExternalInput")
    woT = nc.dram_tensor("woT", [D, C], F32, kind="ExternalInput")
    outp = nc.dram_tensor("outp", [C, HW], F32, kind="ExternalOutput")
    io = (xb, yb, wqT, wkT, wvT, woT, outp)
    with TileContext(nc) as tc:
        for rep in range(repeat):
            _emit_one(nc, tc, io, mm_fast, rep)
    nc.finalize()
    return nc


_NC_CACHE = {}


def _get_nc(mm_fast=True, repeat=1):
    key = (mm_fast, repeat)
    if key not in _NC_CACHE:
        _NC_CACHE[key] = build_nc(mm_fast, repeat)
    return _NC_CACHE[key]


def make_in_maps(x, y, W_qkv, W_out):
    x = np.asarray(x, np.float32).reshape(B, C, HW)
    y = np.asarray(y, np.float32).reshape(B, C, HW)
    W_qkv = np.asarray(W_qkv, np.float32)
    W_out = np.asarray(W_out, np.float32)
    in_maps = []
    for core in range(N_CORES):
        b, h = core // HEADS, core % HEADS
        sl = slice(h * D, (h + 1) * D)
        in_maps.append({
            "xb": np.ascontiguousarray(x[b]),
            "yb": np.ascontiguousarray(y[b]),
            "wqT": np.ascontiguousarray(W_qkv[sl, :].T),
            "wkT": np.ascontiguousarray(W_qkv[HIDDEN + h * D:HIDDEN + (h + 1) * D, :].T),
            "wvT": np.ascontiguousarray(W_qkv[2 * HIDDEN + h * D:2 * HIDDEN + (h + 1) * D, :].T),
            "woT": np.ascontiguousarray(W_out[:, sl].T),
        })
    return in_maps


def gather(results, b_out):
    b_out = np.asarray(b_out, np.float32)
    out = np.zeros((B, C, HW), np.float32)
    for core in range(N_CORES):
        out[core // HEADS] += results[core]["outp"]
    out += b_out[None, :, None]
    return out.reshape(B, C, 64, 64)


def kernel(x, y, W_qkv, W_out, b_out):
    nc = _get_nc(mm_fast=True)
    in_maps = make_in_maps(x, y, W_qkv, W_out)
    res = run_bass_kernel_spmd(nc, in_maps, core_ids=list(range(N_CORES)))
    return gather(res.results, b_out)
